# revision 40
# baseline (speedup 1.0000x reference)
"""ForgetMult linear recurrence h_t = f_t*x_t + (1-f_t)*h_{t-1} on 8 trn2 cores.

Sharding: batch dim B=64 split across 8 cores (8 batches/core). Per core the
(b,h) channels are independent scans over T on the Vector engine
(tensor_tensor_scan, measured 2.0 cyc/elem + 125 cyc overhead, dtype
independent).

I/O is bf16 (harness gate is rel_err < 2e-2; the bf16 pipeline measures
~4e-3 since the scan state stays fp32 internally): 48 MiB/core -> ~140 us
DMA roofline. Host pre-transposes f/x to [B*H, T] bf16 so channel groups
load as [128, T] tiles at line rate (2 KB rows), no PE transposes.

To amortize per-instruction overhead + semaphore sync, 4 channel groups are
chained into ONE scan instruction via separator columns: tile layout
[sep|1024|sep|1024|sep|1024|sep|1024] (W=4100 cols). Separators carry f=1,
x=h0_g, so after the elementwise stages a_sep=1-1=0 and b_sep=1*h0=h0, which
forces state <- 0*state + h0 = h0 at each group boundary -- the scan chains
through all 4 groups in one instruction with exact carry resets.

Per core pipeline per tile (tapered tile sizes [1,1,2,4x14,2,1,1] shrink
pipeline fill/drain; 20 tiles cover 64 groups):
  - DMA in  f,x segments [128, 1024] per group (SP queue); h0 cols into x seps
  - ACT: a = 1 - f over the full [128, 4100] tile (computes a_sep=0 too)
  - DVE: b = f*x in place into the x tile (bf16 2x mode, ~2.3 us)
  - DVE: tensor_tensor_scan over [128, 4100] (~8.7 us; 2 cyc/elem is the
    HW floor for the affine scan, dtype-independent)
  - DMA out 4 segments (ACT queue)
GpSimd stays idle: its ops contend with DVE for the shared SBUF read port and
stretch DVE 2x ops 4-9x (measured), so offloading the TT there loses.
Host upcasts y back to fp32 and restores [T, B, H].

Measured: 195 us HW exec (vs 292 us fp32 baseline), rel err 3.6e-3.
DVE is saturated (scans 139 us + TT 37 us back-to-back, <0.5 us of stalls);
DMA (~140 us) fully hidden. Startup trims: seps via gpsimd memset (h0==0
case; DMA fallback for general h0), a dummy 1-col activation hoists the lazy
ACT_TABLE_LOAD into the DMA fill, first two tiles' x-loads ride the
then-idle scalar queue, and each tile's output DMAs are emitted only after
the NEXT tile's ACT so ACTs never queue behind them on the scalar SEQ.
Stock-instruction floor: going lower needs a custom DVE uOp program fusing
(1-f) and (f*x) into the scan's feed-forward stages (~150 us, the DMA wall).
"""

import ml_dtypes
import numpy as np

import concourse.bacc as bacc
import concourse.bass as bass
import concourse.mybir as mybir
from concourse import bass_utils
from concourse import dve_ops
from concourse.dve_ops import OPS, DveOp
from concourse.dve_spec import Spec, Src0, Src1
from concourse.dve_uop import (
    ENABLE,
    AluInp,
    AluOp,
    DelayInp,
    DveOpSpec,
    InpSel,
    OutPath,
    OutSel,
    Trigger,
    UopConfig,
)
from concourse.tile import TileContext

T = 1024
B = 64
H = 1024
NCORES = 8
BS = B // NCORES  # batches per core
C = BS * H  # channels per core (independent scans)
G = 128  # channels per group == partition dim
NGROUP = C // G  # 64
GPT = 4  # groups chained per scan instruction (tile width cap)
NTILE = NGROUP // GPT  # 16
SEG = T + 1  # 1025: separator column + T timesteps
W = GPT * SEG  # 4100 tile width

F32 = mybir.dt.float32
BF16 = mybir.dt.bfloat16
NPBF16 = ml_dtypes.bfloat16


OP_NAME = "FORGETMULT_SCAN_ANT"


def _register_fused_scan() -> DveOp:
    """Custom DVE op: per element u=1-f; v=f*x; state=u*state+v; out=state.

    Hand-written uOp program modeled on the stock TENSOR_TENSOR_SCAN
    (seed/bubble/compute with the NEXT_ALU_OUT_A state feedback and the
    2-cycle bubble cadence); the 1-f and f*x prep rides the feed-forward
    stages for free, so one instruction replaces ACT + tensor_tensor + scan
    at the same 2 cyc/elem. HW-validated: rel err 1.7e-3 vs fp32 numpy,
    2.27 us per [128,1024] (= stock scan alone)."""
    for op in OPS:
        if op.name == OP_NAME:
            return op
    seed = UopConfig(
        repeat_count=1,
        trigger=(Trigger.COUNT, Trigger.NONE, Trigger.NONE),
        next_uop=(1, 0, 0),
    )
    seed.enable_input(InpSel.CONST_0, 0)
    for k in range(4):
        seed.datapath_config[k].pass_through_alu()
    seed.datapath_config[3].alu_out_a_enable = ENABLE

    bubble = UopConfig(
        repeat_count=1,
        trigger=(Trigger.COUNT, Trigger.NONE, Trigger.NONE),
        next_uop=(2, 0, 0),
    )

    comp = UopConfig(
        repeat_count=1,
        trigger=(Trigger.SRC_TENSOR_DONE, Trigger.COUNT, Trigger.NONE),
        next_uop=(0, 1, 0),
        require_inp0=ENABLE,
        require_inp1=ENABLE,
    )
    comp.enable_input(InpSel.SRC_0, 0)  # f -> ALU lane
    comp.enable_input(InpSel.SRC_1, 1)  # x -> delay chain 0
    comp.enable_input(InpSel.ONE_F32, 2)  # 1.0 -> delay chain 1
    comp.enable_output(OutSel.ALU_OUT, OutPath.WR0_LO)
    dp = comp.datapath_config
    dp[0].enable_alu(AluOp.SUBTRACT, AluInp.PREV_DELAY_1, AluInp.PREV_ALU_OUT)
    dp[0].pass_through_delay(0)
    dp[0].enable_delay_from_src(DelayInp.PREV_ALU_OUT, 1)
    dp[1].enable_alu(AluOp.MULTIPLY, AluInp.PREV_DELAY_1, AluInp.PREV_DELAY_0)
    dp[1].enable_delay_from_src(DelayInp.PREV_ALU_OUT, 0)
    dp[2].enable_alu(AluOp.MULTIPLY, AluInp.PREV_DELAY_0, AluInp.NEXT_ALU_OUT_A)
    dp[2].enable_delay_from_src(DelayInp.PREV_ALU_OUT, 0)
    dp[3].enable_alu(AluOp.ADD, AluInp.PREV_ALU_OUT, AluInp.PREV_DELAY_0)
    dp[3].alu_out_a_enable = ENABLE
    for k in range(4, 8):
        dp[k].pass_through_alu()

    dummy = Spec(
        body=Src0 * Src1,
        reference=lambda in0, in1, s0, s1, imm2: in0 * in1,
    )
    op = DveOp(OP_NAME, dummy, subdim=False, uops_sha={"v3": "cache-seeded"})
    OPS.append(op)
    row = dve_ops._CUSTOM_DVE_ROW_BASE + OPS.index(op)
    dve_ops._SUB_OPCODE_FOR_NAME[OP_NAME] = row
    dve_ops.CUSTOM_DVE_SPECS[OP_NAME] = dummy
    spec = DveOpSpec(name=OP_NAME, opcode=row, uops=[seed, bubble, comp], rd1_en=True)
    spec.validate("v3")
    dve_ops._COMPILE_CACHE[(OP_NAME, "v3")] = spec
    return op


def build_program(h0_is_zero: bool = True) -> bass.Bass:
    fm = _register_fused_scan()
    nc = bacc.Bacc(trn_type="TRN2")
    f_d = nc.dram_tensor("f", (C, T), BF16, kind="ExternalInput")
    x_d = nc.dram_tensor("x", (C, T), BF16, kind="ExternalInput")
    h0_d = nc.dram_tensor("h0", (G, NGROUP), BF16, kind="ExternalInput")
    ones_d = nc.dram_tensor("ones", (G, NTILE * GPT), BF16, kind="ExternalInput")
    y_d = nc.dram_tensor("y", (C, T), BF16, kind="ExternalOutput")

    with TileContext(nc) as tc:
        with (
            tc.tile_pool(name="io", bufs=6) as io,
            tc.tile_pool(name="hpool", bufs=3) as hpool,
        ):
            # Small tiles at the ends shrink pipeline fill/drain: the first
            # scan starts after one group's DMA+ACT+TT (~4 us) instead of
            # four's, and the final output drain is one group (~0.7 us).
            gpts = [1, 1, 2] + [4] * 14 + [2, 1, 1]
            assert sum(gpts) == NGROUP
            g0 = 0
            pending_out = None
            for tl, gpt in enumerate(gpts):
                w = gpt * SEG
                ft = io.tile([G, W], BF16, tag="f")
                xt = io.tile([G, W], BF16, tag="x")
                # separator columns: f=1 -> a_sep=0; x=h0 -> b_sep=h0.
                # With h0==0 (the reference always passes zeros) both seps are
                # constants: fill via ~100ns gpsimd memsets instead of two
                # ~650ns serialized queue DMAs per tile. DMA fallback keeps
                # general-h0 correctness.
                if h0_is_zero:
                    nc.gpsimd.memset(ft[:, 0 : w : SEG], 1.0)
                    nc.gpsimd.memset(xt[:, 0 : w : SEG], 0.0)
                else:
                    nc.sync.dma_start(
                        out=ft[:, 0 : w : SEG], in_=ones_d[:, g0 : g0 + gpt]
                    )
                    nc.sync.dma_start(
                        out=xt[:, 0 : w : SEG], in_=h0_d[:, g0 : g0 + gpt]
                    )
                # Split the input streams across both HWDGE queues: f on
                # sync, x on scalar (which carries only output DMAs now the
                # fused op removed the ACTs). The previous tile's output DMAs
                # are flushed AFTER this tile's x-loads so a y waiting on its
                # scan never blocks the next x-issue on the shared queue.
                for i in range(gpt):
                    rows = slice((g0 + i) * G, (g0 + i + 1) * G)
                    cols = slice(i * SEG + 1, (i + 1) * SEG)
                    nc.sync.dma_start(out=ft[:, cols], in_=f_d[rows, :])
                    nc.scalar.dma_start(out=xt[:, cols], in_=x_d[rows, :])
                if pending_out is not None:
                    pending_out()
                    pending_out = None

                # One fused instruction per tile: u=1-f, v=f*x, state=u*state+v.
                # Separator columns (f=1, x=h0) give u=0, v=h0 so the state
                # resets to h0 at every group boundary; s0 only needs to be
                # finite.
                ht = hpool.tile([G, W], BF16, tag="h")
                nc.vector._custom_dve(
                    fm, out=ht[:, 0:w], in0=ft[:, 0:w], in1=xt[:, 0:w], s0=0.0
                )

                def emit_out(ht=ht, g0=g0, gpt=gpt):
                    for i in range(gpt):
                        rows = slice((g0 + i) * G, (g0 + i + 1) * G)
                        cols = slice(i * SEG + 1, (i + 1) * SEG)
                        nc.scalar.dma_start(out=y_d[rows, :], in_=ht[:, cols])

                pending_out = emit_out
                g0 += gpt
            pending_out()
    if not nc.is_finalized():
        nc.finalize()
    return nc


def run(inputs: dict, trace: bool = False, tmpdir=None) -> tuple[np.ndarray, object]:
    f = np.asarray(inputs["f"], dtype=np.float32)
    x = np.asarray(inputs["x"], dtype=np.float32)
    h0 = np.asarray(inputs["hidden_init"], dtype=np.float32)

    nc = build_program(h0_is_zero=not np.any(h0))

    # [T, B, H] fp32 -> [B*H, T] bf16 once; per-core slices are then
    # contiguous row blocks (zero-copy views).
    fT = np.ascontiguousarray(f.reshape(T, B * H).astype(NPBF16).T)
    xT = np.ascontiguousarray(x.reshape(T, B * H).astype(NPBF16).T)
    ones = np.ones((G, NTILE * GPT), dtype=NPBF16)

    in_maps = []
    for m in range(NCORES):
        rows = slice(m * C, (m + 1) * C)
        h0c = np.ascontiguousarray(
            h0.reshape(B * H)[rows].reshape(NGROUP, G).T.astype(NPBF16)
        )
        in_maps.append({"f": fT[rows], "x": xT[rows], "h0": h0c, "ones": ones})

    res = bass_utils.run_bass_kernel_spmd(
        nc, in_maps, core_ids=list(range(NCORES)), trace=trace, tmpdir=tmpdir
    )
    # y arrives [C, T] bf16 per core; restore [T, BS, H] fp32
    outs = [
        r["y"].reshape(BS, H, T).transpose(2, 0, 1).astype(np.float32)
        for r in res.results
    ]
    return np.concatenate(outs, axis=1), res


def kernel(**inputs) -> np.ndarray:
    out, _ = run(inputs, trace=False)
    return out
